# revision 7
# baseline (speedup 1.0000x reference)
"""AugmentedTripletLoss kernel for 8 Trainium2 NeuronCores.

Strategy (data-parallel over rows, per sharding hint):
  - Each core gets 1024 of the 8192 rows plus a full copy of the inputs
    (the "all-gathered" replica is provided host-side since kernel()
    receives the full arrays).
  - The [1024, 8208] block of D = dist2(i,j) - sq_i + BIG*mask(i,j) is
    computed with TWO accumulated bf16 matmuls per PSUM tile:
       pass1: lhsT = -2*x_rows^T           [128 x 128]
              rhs  = [x^T | cn^T]          [128 x 1024]
       pass2: lhsT = [s*onehot(t_i); 1]    [65  x 128]
              rhs  = [s*onehot(t_j) | 0 ;  [65  x 1024]
                      sq_j           | 1]
    with s^2 = BIG = 4096.  Then per row:
       dist_an^2 = min_j D + sq_i       (diff-class & centers win the min)
       dist_ap^2 = max_j D + sq_i - BIG (same-class entries carry +BIG)
    and loss_row = relu(dist_ap - dist_an + margin).
  - Row-local hard mining = one min+max reduction per PSUM block, spread
    over DVE (reduce), GpSimd (running elementwise max) and ACT (PSUM->
    SBUF staging).  Per-core partial row-loss sums are returned and
    averaged on the host (the "all-reduce mean").
"""

import numpy as np

N, D, NCTR, C = 8192, 128, 16, 64
NCORES = 8
RPC = N // NCORES          # rows per core = 1024
MT = RPC // 128            # m-tiles per core = 8
NCOL = N + NCTR            # 8208 columns (samples + centers)
NBLK = 8                   # full [128,1024] column blocks per m-tile
BIG = 4096.0
S = 64.0                   # sqrt(BIG)
MARGIN = 1.0
EPS = 1e-12
POOLMAX_BLOCKS = ()   # GpSimd TT is not a legal Pool ISA op on V3; keep reduces on DVE

_CACHE = {}


def _build_program():
    from concourse import bacc, mybir, tile
    from concourse.bass import ts

    f32 = mybir.dt.float32
    bf16 = mybir.dt.bfloat16
    X = mybir.AxisListType.X
    XY = mybir.AxisListType.XY
    Alu = mybir.AluOpType
    Act = mybir.ActivationFunctionType

    nc = bacc.Bacc(
        "TRN2", target_bir_lowering=False, debug=False, enable_asserts=False
    )

    xT_d = nc.dram_tensor("xT", [D, N], bf16, kind="ExternalInput").ap()
    xcT_d = nc.dram_tensor("xcoreT", [D, RPC], bf16, kind="ExternalInput").ap()
    xc_d = nc.dram_tensor("xcore", [RPC, D], f32, kind="ExternalInput").ap()
    rhs2_d = nc.dram_tensor("rhs2f", [C + 1, NCOL], bf16, kind="ExternalInput").ap()
    lhs2_d = nc.dram_tensor("lhs2", [C + 1, RPC], bf16, kind="ExternalInput").ap()
    ctr_d = nc.dram_tensor("center", [NCTR, D], f32, kind="ExternalInput").ap()
    id_d = nc.dram_tensor("ident", [NCTR, NCTR], bf16, kind="ExternalInput").ap()
    ones_d = nc.dram_tensor("ones128", [128, 128], bf16, kind="ExternalInput").ap()
    out_d = nc.dram_tensor("out", [1, 1], f32, kind="ExternalOutput").ap()

    with tile.TileContext(nc) as tc:
        with (
            tc.tile_pool(name="per", bufs=1) as per,
            tc.tile_pool(name="xsqp", bufs=2) as xsqp,
            tc.tile_pool(name="cp", bufs=4) as cp,
            tc.tile_pool(name="accp", bufs=2) as accp,
        ):
            # ---- persistent SBUF tensors ----
            rhs1s = per.tile([D, NCOL], bf16, tag="rhs1s")
            rhs2s = per.tile([C + 1, NCOL], bf16, tag="rhs2s")
            lhs1s = per.tile([D, RPC], bf16, tag="lhs1s")
            lhs2s = per.tile([C + 1, RPC], bf16, tag="lhs2s")
            xcts = per.tile([D, RPC], bf16, tag="xcts")
            xcs = per.tile([128, MT, D], f32, tag="xcs")
            xcsq = per.tile([128, MT, D], f32, tag="xcsq")
            sqi = per.tile([128, MT], f32, tag="sqi")
            mins = per.tile([128, MT, NBLK + 1], f32, tag="mins")
            maxs = per.tile([128, MT, NBLK + 2], f32, tag="maxs")
            ctrs = per.tile([NCTR, D], f32, tag="ctrs")
            cns = per.tile([NCTR, D], bf16, tag="cns")
            idents = per.tile([NCTR, NCTR], bf16, tag="idents")
            ones128 = per.tile([128, 128], bf16, tag="ones128")
            onescol = per.tile([128, 1], f32, tag="onescol")
            outs = per.tile([1, 1], f32, tag="outs")
            pos2 = per.tile([128, MT], f32, tag="pos2")
            neg2 = per.tile([128, MT], f32, tag="neg2")
            apd = per.tile([128, MT], f32, tag="apd")
            andt = per.tile([128, MT], f32, tag="andt")
            rl = per.tile([128, MT], f32, tag="rl")
            rsum = per.tile([128, 1], f32, tag="rsum")

            # ---- input DMAs ----
            for i in range(8):
                nc.sync.dma_start(
                    out=rhs1s[:, ts(i, 1024)], in_=xT_d[:, ts(i, 1024)]
                )
            for i in range(2):
                nc.sync.dma_start(
                    out=rhs2s[:, i * 4104 : (i + 1) * 4104],
                    in_=rhs2_d[:, i * 4104 : (i + 1) * 4104],
                )
            nc.sync.dma_start(out=xcts[:, :], in_=xcT_d[:, :])
            nc.sync.dma_start(
                out=xcs[:, :, :], in_=xc_d.rearrange("(t p) d -> p t d", p=128)
            )
            nc.sync.dma_start(out=lhs2s[:, :], in_=lhs2_d[:, :])
            nc.sync.dma_start(out=ctrs[:, :], in_=ctr_d[:, :])
            nc.sync.dma_start(out=idents[:, :], in_=id_d[:, :])
            nc.sync.dma_start(out=ones128[:, :], in_=ones_d[:, :])

            nc.vector.memset(onescol[:, :], 1.0)
            nc.vector.memset(maxs[:, :, :], -3.0e38)

            # ---- prep: lhs1 = -2 * xcoreT ----
            nc.vector.tensor_scalar_mul(lhs1s[:, :], xcts[:, :], -2.0)

            # ---- prep: per-row sq_i (exact fp32) ----
            nc.scalar.square(xcsq[:, :, :], xcs[:, :, :])
            nc.vector.tensor_reduce(sqi[:, :], xcsq[:, :, :], X, Alu.add)

            # ---- prep: normalized centers ----
            csq = per.tile([NCTR, D], f32, tag="csq")
            cn2 = per.tile([NCTR, 1], f32, tag="cn2")
            cnr = per.tile([NCTR, 1], f32, tag="cnr")
            cni = per.tile([NCTR, 1], f32, tag="cni")
            nc.scalar.square(csq[:, :], ctrs[:, :])
            nc.vector.tensor_reduce(cn2[:, :], csq[:, :], X, Alu.add)
            nc.scalar.sqrt(cnr[:, :], cn2[:, :])
            nc.vector.reciprocal(cni[:, :], cnr[:, :])
            nc.vector.tensor_scalar(
                out=cns[:, :], in0=ctrs[:, :], scalar1=cni[:, :], scalar2=None,
                op0=Alu.mult,
            )

            # ---- prep: center transpose + sq row -> rhs2s[64, 0:8192] ----
            with tc.tile_pool(name="sp", bufs=1, space="PSUM") as sp:
                ctp = sp.tile([128, 2048], bf16, tag="sqpt")
                nc.tensor.transpose(ctp[:, 0:NCTR], cns[:, :], idents[:, :])
                nc.scalar.copy(rhs1s[:, N : N + NCTR], ctp[:, 0:NCTR])
                for r in range(8):
                    cols = slice(r * 1024, (r + 1) * 1024)
                    xsqt = xsqp.tile([128, 1024], bf16, tag="xsqt")
                    nc.vector.tensor_tensor(
                        out=xsqt[:, :], in0=rhs1s[:, cols], in1=rhs1s[:, cols],
                        op=Alu.mult,
                    )
                    pt = sp.tile([128, 1024], f32, tag="sqpt")
                    for h in range(2):
                        nc.tensor.matmul(
                            pt[:, ts(h, 512)],
                            ones128[:, :],
                            xsqt[:, ts(h, 512)],
                            start=True,
                            stop=True,
                        )
                    nc.scalar.copy(rhs2s[64:65, cols], pt[64:65, :])

            # ---- main sweep ----
            with tc.tile_pool(name="pp", bufs=3, space="PSUM") as pp:
                for m in range(MT):
                    w1 = lhs1s[:, ts(m, 128)]
                    w2 = lhs2s[:, ts(m, 128)]
                    acc = accp.tile([128, 1024], f32, tag="acc")
                    first_pool = True
                    for b in range(NBLK + 1):
                        w = 1024 if b < NBLK else NCTR
                        nh = 2 if b < NBLK else 1
                        pt = pp.tile([128, 1024], f32, tag="ptile")
                        for h in range(nh):
                            hw = min(512, w)
                            nc.tensor.matmul(
                                pt[:, h * 512 : h * 512 + hw],
                                w1,
                                rhs1s[:, 1024 * b + 512 * h : 1024 * b + 512 * h + hw],
                                start=True,
                                stop=False,
                            )
                        for h in range(nh):
                            hw = min(512, w)
                            nc.tensor.matmul(
                                pt[:, h * 512 : h * 512 + hw],
                                w2,
                                rhs2s[:, 1024 * b + 512 * h : 1024 * b + 512 * h + hw],
                                start=False,
                                stop=True,
                            )
                        if b < NBLK:
                            inap = pt[:, :].rearrange("p (u v) -> p u v", v=512)
                            red = XY
                        else:
                            inap = pt[:, :w]
                            red = X
                        nc.vector.tensor_reduce(
                            mins[:, m, b : b + 1], inap, red, Alu.min
                        )
                        if b < NBLK and b in POOLMAX_BLOCKS:
                            ct = cp.tile([128, 1024], f32, tag="ct")
                            nc.scalar.copy(ct[:, :], pt[:, :])
                            if first_pool:
                                nc.gpsimd.tensor_tensor(
                                    out=acc[:, :], in0=ct[:, :], in1=ct[:, :],
                                    op=Alu.max,
                                )
                                first_pool = False
                            else:
                                nc.gpsimd.tensor_tensor(
                                    out=acc[:, :], in0=acc[:, :], in1=ct[:, :],
                                    op=Alu.max,
                                )
                        else:
                            nc.vector.tensor_reduce(
                                maxs[:, m, b : b + 1], inap, red, Alu.max
                            )
                    if not first_pool:
                        nc.vector.tensor_reduce(
                            maxs[:, m, NBLK + 1 : NBLK + 2],
                            acc[:, :].rearrange("p (u v) -> p u v", v=512),
                            XY,
                            Alu.max,
                        )

                # ---- epilogue (vectorized over the 8 m-tiles) ----
                posr = per.tile([128, MT], f32, tag="posr")
                negr = per.tile([128, MT], f32, tag="negr")
                nc.vector.tensor_reduce(posr[:, :], maxs[:, :, :], X, Alu.max)
                nc.vector.tensor_reduce(negr[:, :], mins[:, :, :], X, Alu.min)

                nc.vector.tensor_tensor(
                    out=pos2[:, :], in0=posr[:, :], in1=sqi[:, :], op=Alu.add
                )
                nc.vector.tensor_scalar(
                    out=pos2[:, :], in0=pos2[:, :], scalar1=BIG, scalar2=EPS,
                    op0=Alu.subtract, op1=Alu.max,
                )
                nc.scalar.sqrt(apd[:, :], pos2[:, :])

                nc.vector.tensor_tensor(
                    out=neg2[:, :], in0=negr[:, :], in1=sqi[:, :], op=Alu.add
                )
                nc.vector.tensor_scalar(
                    out=neg2[:, :], in0=neg2[:, :], scalar1=EPS, scalar2=None,
                    op0=Alu.max,
                )
                nc.scalar.sqrt(andt[:, :], neg2[:, :])

                nc.vector.tensor_tensor(
                    out=rl[:, :], in0=apd[:, :], in1=andt[:, :], op=Alu.subtract
                )
                nc.scalar.activation(rl[:, :], rl[:, :], Act.Relu, bias=MARGIN)
                nc.vector.tensor_reduce(rsum[:, :], rl[:, :], X, Alu.add)

                fin = pp.tile([128, 1024], f32, tag="ptile")
                nc.tensor.matmul(
                    fin[0:1, 0:1], onescol[:, :], rsum[:, :], start=True, stop=True
                )
                nc.scalar.copy(outs[:, :], fin[0:1, 0:1])
                nc.sync.dma_start(out=out_d[:, :], in_=outs[:, :])

    nc.compile()
    return nc


def _make_in_maps(inputs, targets, center):
    import ml_dtypes

    bf = ml_dtypes.bfloat16
    x = np.ascontiguousarray(np.asarray(inputs, dtype=np.float32))
    t = np.asarray(targets).astype(np.int64)
    c = np.ascontiguousarray(np.asarray(center, dtype=np.float32))
    xT = np.ascontiguousarray(x.T).astype(bf)
    oh = ((t[None, :] == np.arange(C)[:, None]).astype(np.float32) * S).astype(bf)
    rhs2f = np.zeros((C + 1, NCOL), dtype=bf)
    rhs2f[:C, :N] = oh
    rhs2f[C, N:] = np.ones((NCTR,), dtype=bf)
    ident = np.eye(NCTR, dtype=np.float32).astype(bf)
    in_maps = []
    for k in range(NCORES):
        rows = slice(RPC * k, RPC * (k + 1))
        lhs2 = np.concatenate(
            [oh[:, rows], np.ones((1, RPC), dtype=bf)], axis=0
        )
        in_maps.append(
            {
                "xT": xT,
                "xcoreT": np.ascontiguousarray(xT[:, rows]),
                "xcore": np.ascontiguousarray(x[rows]),
                "rhs2f": rhs2f,
                "lhs2": np.ascontiguousarray(lhs2),
                "center": c,
                "ident": ident,
                "ones128": np.ones((128, 128), dtype=bf),
            }
        )
    return in_maps


def run(inputs, targets, center, trace=False, tmpdir=None):
    """Returns (loss_scalar, BassKernelResults)."""
    from concourse.bass_utils import run_bass_kernel_spmd

    if "nc" not in _CACHE:
        _CACHE["nc"] = _build_program()
    nc = _CACHE["nc"]
    in_maps = _make_in_maps(inputs, targets, center)
    res = run_bass_kernel_spmd(
        nc, in_maps, list(range(NCORES)), trace=trace, tmpdir=tmpdir
    )
    total = sum(float(r["out"][0, 0]) for r in res.results)
    loss = np.array(total / N, dtype=np.float32)
    return loss, res


def kernel(inputs, targets, center):
    loss, _ = run(inputs, targets, center, trace=False)
    return loss
